# revision 48
# baseline (speedup 1.0000x reference)
"""Trainium2 Bass kernel for nn_MultiHeadSelfAttentionBlock.

Data-parallel over batch (B=32 -> 4 per core on 8 cores). Single-core
pipeline, bf16 matmul operands (fp32 PSUM accumulation) throughout:

  - All weight preprocessing happens on host (numpy): BN folded to
    per-channel scale/shift, q/out projections transposed into lhsT
    layout, k/v 1x1 projections merged with the depthwise-conv taps and
    k/v BN into 45 per-(chunk,tap) [128c, 64kd_k||64kd_v] bf16 blocks
    (k-side prescaled by 1/sqrt(64)), BN-shift constants reduced to a
    [128,1] vector, layer-scale replicated to a [8,1024] row table.
  - Per item: x loaded once (kept fp32 for the residual); BN applied on
    GPSIMD writing bf16 into a zero-padded [c, 34x34] buffer.  q proj
    reads 128-pixel slices of the padded buffer as the stationary
    operand; conv taps read strided 16x16/stride-2 windows as the
    moving operand -- no im2col staging, and k+v share each matmul.
  - Logits computed transposed [p, l] per head (the torch .view
    head-split bug resolves to l = 16*c + 2*t + par, kd = s_lo); the
    softmax denominator comes free as row 64 of the o-matmul via a ones
    column appended to V^T.  exp on Scalar (bf16 out), denominator rows
    copied to SBUF on Scalar, one reciprocal per item on DVE with
    layer-scale folded in, broadcast across partitions via a DRAM
    bounce per head-pair, normalize on GPSIMD.
  - Output projection accumulates in PSUM; epilogue is a single DVE add
    of the fp32 residual.  Emission is software-pipelined: item b-1's
    output projection is emitted between taps(b) and attention(b) so
    the PE never idles on the denominator DRAM round-trip.
"""

from contextlib import ExitStack

import os

import numpy as np

import concourse.bacc as bacc
import concourse.bass as bass
import concourse.tile as tile
from concourse import mybir
from concourse.masks import make_identity
from concourse.dve_ops import RECIPROCAL_APPROX_FAST, RECIP_APPROX_FAST_CONSTS

F32 = mybir.dt.float32
BF16 = mybir.dt.bfloat16
F8 = mybir.dt.float8e4
DR = mybir.MatmulPerfMode.DoubleRow
ALU = mybir.AluOpType
ACTF = mybir.ActivationFunctionType
LS_EXP = 23                  # o_norm carries ls * 2^23 to stay in fp8 range
OW_EXP = 8                   # out_w scaled by 2^8 to avoid fp8 denormals

B, C, H, W = 32, 640, 32, 32
NH, KD, VD = 8, 64, 64
S = H * W            # 1024
P = 256              # key/value positions (16x16)
EPS = 1e-3
N_CORES = 8
BPC = B // N_CORES   # 4 batch items per core
NCH = C // 128       # 5 channel chunks
PW = 34              # padded image width
PSZ = PW * PW        # 1156


def _fap(base, free_off, dims):
    """AP with base's partition dim and explicit free dims [[step, count],...]."""
    return bass.AP(tensor=base.tensor, offset=base.offset + free_off,
                   ap=[base.ap[0]] + dims)


def build_nc():
    nc = bacc.Bacc(None, target_bir_lowering=False, debug=False)

    x4 = nc.dram_tensor("x", [BPC, C, H, W], F32, kind="ExternalInput")
    qwT_d = nc.dram_tensor("p_qwT", [128, NCH * 512], BF16, kind="ExternalInput")
    wtap_d = nc.dram_tensor("p_wtap", [128, NCH * 9 * 128], BF16,
                            kind="ExternalInput")
    owT_d = nc.dram_tensor("p_owT", [128, 4 * C], F8, kind="ExternalInput")
    kvc_d = nc.dram_tensor("p_kvconst", [128, 1], F32, kind="ExternalInput")
    bnio_d = nc.dram_tensor("p_bnio", [128, 2 * NCH], F32, kind="ExternalInput")
    lsr_d = nc.dram_tensor("p_lsrow", [128, 2 * S], BF16, kind="ExternalInput")
    out4 = nc.dram_tensor("out", [BPC, C, H, W], F32, kind="ExternalOutput")
    KSTAGE = int(os.environ.get("KSTAGE", "99"))

    with tile.TileContext(nc) as tc, ExitStack() as ctx:
        wp = ctx.enter_context(tc.tile_pool(name="wp", bufs=1))
        # single PSUM pool, tags sized to exactly 8 banks:
        #   mm 2x[128,512]f32 + kvf 1x[128,256]f32 + lg 2x[128,512]f32
        #   + op 3x[65,512]f32
        pp = ctx.enter_context(tc.tile_pool(name="pp", bufs=1, space="PSUM"))
        xin = ctx.enter_context(tc.tile_pool(name="xin", bufs=2 * NCH))
        xnfp = ctx.enter_context(tc.tile_pool(name="xnfp", bufs=NCH))
        qbp = ctx.enter_context(tc.tile_pool(name="qbp", bufs=2))
        ep = ctx.enter_context(tc.tile_pool(name="ep", bufs=4))
        kvp = ctx.enter_context(tc.tile_pool(name="kvp", bufs=2))
        orp = ctx.enter_context(tc.tile_pool(name="orp", bufs=8))
        onp = ctx.enter_context(tc.tile_pool(name="onp", bufs=4))
        rbcp = ctx.enter_context(tc.tile_pool(name="rbcp", bufs=4))
        dap = ctx.enter_context(tc.tile_pool(name="dap", bufs=2))
        osb = ctx.enter_context(tc.tile_pool(name="osb", bufs=2))
        drp = ctx.enter_context(tc.tile_pool(name="drp", bufs=2, space="DRAM"))

        # ---------------- setup ----------------
        identf = wp.tile([64, 64], F32, tag="identf", name="identf")
        make_identity(nc, identf[:])
        ones1 = wp.tile([128, 1], BF16, tag="ones1", name="ones1")
        nc.gpsimd.memset(ones1[:], 1.0)

        qwT = wp.tile([128, NCH * 512], BF16, tag="qwT", name="qwT")
        nc.sync.dma_start(out=qwT[:], in_=qwT_d[:, :])
        wtap = wp.tile([128, NCH * 9 * 128], BF16, tag="wtap", name="wtap")
        nc.sync.dma_start(out=wtap[:], in_=wtap_d[:, :])
        owT = wp.tile([128, 4 * C], F8, tag="owT", name="owT")
        nc.sync.dma_start(out=owT[:], in_=owT_d[:, :])
        kvc = wp.tile([128, 1], F32, tag="kvc", name="kvc")
        nc.sync.dma_start(out=kvc[:], in_=kvc_d[:, :])
        bnio = wp.tile([128, 2 * NCH], F32, tag="bnio", name="bnio")
        nc.sync.dma_start(out=bnio[:], in_=bnio_d[:, :])
        lsrow = wp.tile([128, 2 * S], BF16, tag="lsrow", name="lsrow")
        nc.sync.dma_start(out=lsrow[:], in_=lsr_d[:, :])

        # denominator staging: head n lives at partition 32*(n%4), column
        # block S*(n//4) (engines only address start partitions 0/32/64/96).
        dall_t = [dap.tile([128, 2 * S], F32, tag="dall", name="dall")
                  for _ in range(2)]
        for i in range(2):
            nc.gpsimd.memset(dall_t[i][:], 1.0)

        # zero-padded xn buffers: 2 item-slots x NCH chunks; borders are
        # zeroed once here and only the 32x32 interior is rewritten per item.
        xnpad = [[wp.tile([128, PSZ], BF16, tag=f"xnp{i}_{ch}",
                          name=f"xnp{i}_{ch}")
                  for ch in range(NCH)] for i in range(2)]
        for i in range(2):
            for ch in range(NCH):
                nc.gpsimd.memset(xnpad[i][ch][:], 0.0)

        def xn_interior(t):
            """interior write AP: [128, 32, 32] at offset (1,1) of 34x34."""
            return _fap(t, PW + 1, [[PW, H], [1, W]])

        def xn_tap(t, dy, dx):
            """moving conv-tap window: stride-2 16x16 -> [128c, 256p]."""
            return _fap(t, PW * dy + dx, [[2 * PW, 16], [2, 16]])

        prev = None  # (b, o_norm tiles, x tiles)

        def emit_outproj(bp, onorm_p, xt_p):
            for ch in range(NCH):
                ot = osb.tile([128, S], F32, tag="outsb", name="outsb")
                for sh in range(2):
                    po = pp.tile([128, 512], F32, tag="mm", bufs=2, name="po")
                    for j in range(2):   # DoubleRow over nv-chunk pairs
                        lhsT = bass.AP(
                            tensor=owT.tensor,
                            offset=owT.offset + C * 2 * j + 128 * ch,
                            ap=[owT.ap[0], [C, 2], [1, 128]])
                        rhs = _fap(onorm_p[j][:], 512 * sh,
                                   [[S, 2], [1, 512]])
                        nc.tensor.matmul(po[:], lhsT, rhs, perf_mode=DR,
                                         start=(j == 0), stop=(j == 1))
                    sl = slice(512 * sh, 512 * (sh + 1))
                    nc.vector.scalar_tensor_tensor(
                        out=ot[:, sl], in0=po[:],
                        scalar=float(2.0 ** -(LS_EXP + OW_EXP)),
                        in1=xt_p[ch][:, sl], op0=ALU.mult, op1=ALU.add)
                nc.sync.dma_start(
                    out=out4[bp, 128 * ch:128 * (ch + 1), :, :].rearrange(
                        "c h w -> c (h w)"),
                    in_=ot[:])

        # ================= per batch item =================
        for b in range(BPC):
            slot = b % 2
            # ---- load x (kept for residual), BN -> flat + padded bf16 ----
            xts, xnfs = [], []
            for ch in range(NCH):
                xt = xin.tile([128, S], F32, tag="xin", name="xin")
                nc.sync.dma_start(
                    out=xt[:],
                    in_=x4[b, 128 * ch:128 * (ch + 1), :, :].rearrange(
                        "c h w -> c (h w)"))
                xts.append(xt)
            for ch in range(NCH):
                xnf = xnfp.tile([128, S], BF16, tag="xnf", name="xnf")
                nc.gpsimd.tensor_scalar(
                    out=xnf[:], in0=xts[ch][:],
                    scalar1=bnio[:, ch:ch + 1], scalar2=bnio[:, NCH + ch:NCH + ch + 1],
                    op0=ALU.mult, op1=ALU.add)
                nc.scalar.activation(xn_interior(xnpad[slot][ch]), xnf[:],
                                     ACTF.Copy)
                xnfs.append(xnf)

            # ---- q projection -> qbuf [s%128, 512*t + c] (t-major) ----
            qbuf = qbp.tile([128, 8 * 512], BF16, tag="qbuf", name="qbuf")
            for t in range(8):
                qp = pp.tile([128, 512], F32, tag="mm", bufs=2, name="qp")
                for ch in range(NCH):
                    nc.tensor.matmul(qp[:], xnfs[ch][:, 128 * t:128 * (t + 1)],
                                     qwT[:, 512 * ch:512 * (ch + 1)],
                                     start=(ch == 0), stop=(ch == NCH - 1))
                nc.vector.tensor_copy(qbuf[:, 512 * t:512 * (t + 1)], qp[:])

            if KSTAGE == 1:
                nc.sync.dma_start(
                    out=out4[b, 0:128, :, :].rearrange("c h w -> c (h w)"),
                    in_=qbuf[:, 0:1024].bitcast(F32))
                continue

            # ---- merged k|v conv taps -> kvf PSUM [64kf || 64vf, 256] ----
            kvf = pp.tile([128, 256], F32, tag="mm", bufs=2, name="kvf")
            for ch in range(NCH):
                for t in range(9):
                    nc.tensor.matmul(
                        kvf[:],
                        wtap[:, 128 * (9 * ch + t):128 * (9 * ch + t + 1)],
                        xn_tap(xnpad[slot][ch], t // 3, t % 3),
                        start=(ch == 0 and t == 0),
                        stop=(ch == NCH - 1 and t == 8))
            kfdup = kvp.tile([128, 256], BF16, tag="f_k", name="f_k")
            nc.vector.tensor_scalar_add(kfdup[0:64, :], kvf[0:64, :],
                                        kvc[0:64, :])
            nc.vector.tensor_scalar_add(kfdup[64:128, :], kvf[0:64, :],
                                        kvc[0:64, :])
            vf = kvp.tile([64, 256], F32, tag="f_v", name="f_v")
            nc.vector.tensor_scalar_add(vf[:], kvf[64:128, :], kvc[64:128, :])

            # V'^T with ones column, fp8; p-tiles at 16-aligned stride 80
            # (DoubleRow requires the pair-dim step % 16 == 0)
            vT8 = kvp.tile([128, 2 * 80], F8, tag="vT8", name="vT8")
            for pt in range(2):
                tp = pp.tile([128, 512], F32, tag="mm", bufs=2, name="tp")
                nc.tensor.transpose(tp[:128, 0:64],
                                    vf[:, 128 * pt:128 * (pt + 1)],
                                    identf[0:64, 0:64])
                nc.scalar.activation(vT8[:, 80 * pt:80 * pt + 64],
                                     tp[:128, 0:64], ACTF.Copy)
                nc.vector.tensor_copy(vT8[:, 80 * pt + 64:80 * pt + 65],
                                      ones1[:])

            if KSTAGE == 2:
                nc.sync.dma_start(
                    out=out4[b, 0:128, 0:4, :].rearrange("c h w -> c (h w)"),
                    in_=kfdup[:, :].bitcast(F32))
                continue

            # ---- previous item's output projection (pipelined) ----
            if prev is not None:
                emit_outproj(*prev)
                prev = None

            # ---- attention ----
            dall = dall_t[slot]
            o_resh = [orp.tile([128, S], BF16, tag="oresh", name="oresh")
                      for _ in range(4)]
            rec = dap.tile([128, 2 * S], F32, tag="rec", bufs=1, name="rec")
            dall2 = dap.tile([128, 2 * S], BF16, tag="dall2", name="dall2")
            dscr = drp.tile([NH, S], BF16, tag="dscr", name="dscr")
            o_norm = []
            rbcs = []

            def denom_halfbatch(blk):
                # reciprocal + layer scale for heads 4*blk..4*blk+3, then
                # bounce to DRAM and normalize the two finished c2 blocks
                cs = slice(S * blk, S * (blk + 1))
                nc.vector._custom_dve(
                    RECIPROCAL_APPROX_FAST, out=rec[:, cs], in0=dall[:, cs],
                    s0=RECIP_APPROX_FAST_CONSTS["s0"],
                    s1=RECIP_APPROX_FAST_CONSTS["s1"],
                    imm2=RECIP_APPROX_FAST_CONSTS["imm2"])
                nc.vector.tensor_tensor(out=dall2[:, cs], in0=rec[:, cs],
                                        in1=lsrow[:, cs], op=ALU.mult)
                for m in range(4):
                    nc.sync.dma_start(
                        out=dscr[4 * blk + m:4 * blk + m + 1, :],
                        in_=dall2[32 * m:32 * m + 1, cs])
                # one fp8 pair-tile per half: cols [0:S] = c2 even, [S:2S] odd
                on = onp.tile([128, 2 * S], F8, tag="onorm", name="onorm")
                for c2 in (2 * blk, 2 * blk + 1):
                    rbc = rbcp.tile([128, S], BF16, tag="rbc", name="rbc")
                    rbcs.append(rbc)
                    nc.sync.dma_start(
                        out=rbc[:],
                        in_=bass.AP(tensor=dscr.tensor,
                                    offset=dscr.offset + S * 2 * c2,
                                    ap=[[S, 2], [0, 64], [1, S]]))
                    nc.vector.tensor_tensor(
                        out=on[:, S * (c2 % 2):S * (c2 % 2 + 1)],
                        in0=o_resh[c2][:], in1=rbc[:], op=ALU.mult)
                o_norm.append(on)

            for n in range(NH):
                E8 = ep.tile([128, 2 * S], F8, tag="E", name="E")
                for pt in range(2):
                    for par in range(2):
                        lg = pp.tile([128, 512], F32, tag="lg", bufs=2,
                                     name="lg")
                        # head n's q: cols {512*t + 64*n + c'} of qbuf
                        rhs = _fap(qbuf[64 * par:64 * (par + 1)], 64 * n,
                                   [[512, 8], [1, 64]])
                        nc.tensor.matmul(
                            lg[:],
                            kfdup[64 * par:64 * (par + 1),
                                  128 * pt:128 * (pt + 1)],
                            rhs, start=True, stop=True)
                        nc.scalar.activation(
                            E8[:, S * pt + 512 * par:S * pt + 512 * (par + 1)],
                            lg[:], ACTF.Exp)
                # o via DoubleRow over the two p-tiles: op_t cols =
                # 512*par + j', j' = 64*t + c' (l = 16*c' + 2*t + par)
                op_t = pp.tile([65, 2 * 512], F32, tag="op", bufs=2, name="op")
                for par in range(2):
                    lhsT = bass.AP(tensor=vT8.tensor, offset=vT8.offset,
                                   ap=[vT8.ap[0], [80, 2], [1, 65]])
                    nc.tensor.matmul(
                        op_t[:, 512 * par:512 * (par + 1)], lhsT,
                        _fap(E8[:], 512 * par, [[S, 2], [1, 512]]),
                        perf_mode=DR, start=True, stop=True)
                # un-permute to s-order on the write: src col 512*par + j
                # -> dst col 2*j + par  (s = l = 2*j + par)
                nc.scalar.activation(
                    _fap(dall[32 * (n % 4):32 * (n % 4) + 1], S * (n // 4),
                         [[1, 2], [2, 512]]),
                    op_t[64:65, :], ACTF.Copy)
                nc.vector.tensor_copy(
                    _fap(o_resh[n // 2][64 * (n % 2):64 * (n % 2) + 64], 0,
                         [[1, 2], [2, 512]]),
                    op_t[0:64, :])
                if n % 4 == 3:
                    denom_halfbatch(n // 4)

            if KSTAGE == 4:
                nc.sync.dma_start(
                    out=out4[b, 0:128, 0:16, :].rearrange("c h w -> c (h w)"),
                    in_=dall[:, 0:512])
                nc.sync.dma_start(
                    out=out4[b, 0:128, 16:32, :].rearrange("c h w -> c (h w)"),
                    in_=dall[:, 1024:1536])
                nc.sync.dma_start(
                    out=out4[b, 128:256, 0:16, :].rearrange("c h w -> c (h w)"),
                    in_=rec[:, 0:512])
                nc.sync.dma_start(
                    out=out4[b, 128:256, 16:32, :].rearrange("c h w -> c (h w)"),
                    in_=rec[:, 1024:1536])
                nc.sync.dma_start(
                    out=out4[b, 256:384, 0:16, :].rearrange("c h w -> c (h w)"),
                    in_=o_resh[1][:, :].bitcast(F32))
                nc.sync.dma_start(
                    out=out4[b, 384:512, 0:32, :].rearrange("c h w -> c (h w)"),
                    in_=dall2[:, :].bitcast(F32))
                nc.sync.dma_start(
                    out=out4[b, 512:640, 0:16, :].rearrange("c h w -> c (h w)"),
                    in_=rbcs[1][:, :].bitcast(F32))
                continue
            if KSTAGE == 3:
                for j in range(2):
                    nc.sync.dma_start(
                        out=out4[b, 128 * j:128 * (j + 1), 0:16, :].rearrange(
                            "c h w -> c (h w)"),
                        in_=o_norm[j][:, :].bitcast(F32))
                continue
            prev = (b, o_norm, xts)

        if prev is not None:
            emit_outproj(*prev)

    nc.finalize()
    return nc


def _pack_inputs(inputs):
    """Host-side weight folding: everything that doesn't depend on x."""
    import ml_dtypes

    f32 = lambda n: np.asarray(inputs[n], dtype=np.float32)
    bnf = {}
    for p in ("in", "k", "v"):
        sc = f32(f"{p}_bn_gamma") / np.sqrt(f32(f"{p}_bn_var") + EPS)
        sh = f32(f"{p}_bn_beta") - f32(f"{p}_bn_mean") * sc
        bnf[p] = (sc, sh)

    def tobf(a):
        return np.ascontiguousarray(a.astype(ml_dtypes.bfloat16))

    def tof8(a):
        return np.ascontiguousarray(a.astype(mybir.dt.np(mybir.dt.float8e4)))

    q_w = f32("q_w")                     # [512, 640]
    qwT = np.zeros((128, NCH * 512), np.float32)
    for ch in range(NCH):
        qwT[:, 512 * ch:512 * (ch + 1)] = q_w[:, 128 * ch:128 * (ch + 1)].T

    sck, shk = bnf["k"]
    scv, shv = bnf["v"]
    kw_s = f32("k_w") * sck[None, :] * 0.125      # [64, 640]
    vw_s = f32("v_w") * scv[None, :]
    kdw = f32("k_dw_w").reshape(C, 9)
    vdw = f32("v_dw_w").reshape(C, 9)
    wtap = np.zeros((128, NCH * 9 * 128), np.float32)
    for ch in range(NCH):
        cs = slice(128 * ch, 128 * (ch + 1))
        for t in range(9):
            blk = wtap[:, 128 * (9 * ch + t):128 * (9 * ch + t + 1)]
            blk[:, 0:64] = kw_s[:, cs].T * kdw[cs, t][:, None]
            blk[:, 64:128] = vw_s[:, cs].T * vdw[cs, t][:, None]

    kvconst = np.zeros((128, 1), np.float32)
    kvconst[0:64, 0] = (kw_s @ shk)
    kvconst[64:128, 0] = (vw_s @ shv)

    out_w = f32("out_w") * float(2.0 ** 8)   # fp8-friendly scale
    owT = np.zeros((128, 4 * C), np.float32)
    for nv in range(4):
        owT[:, C * nv:C * (nv + 1)] = out_w[:, 128 * nv:128 * (nv + 1)].T

    sci, shi = bnf["in"]
    bnio = np.zeros((128, 2 * NCH), np.float32)
    for ch in range(NCH):
        bnio[:, ch] = sci[128 * ch:128 * (ch + 1)]
        bnio[:, NCH + ch] = shi[128 * ch:128 * (ch + 1)]

    ls = f32("ls_gamma") * float(2.0 ** 23)  # fp8-friendly scale for o_norm
    lsrow = np.zeros((128, 2 * S), np.float32)
    for n in range(NH):
        lsrow[32 * (n % 4), S * (n // 4):S * (n // 4 + 1)] = np.tile(ls, H)

    return {
        "p_qwT": tobf(qwT),
        "p_wtap": tobf(wtap),
        "p_owT": tof8(owT),
        "p_kvconst": np.ascontiguousarray(kvconst),
        "p_bnio": np.ascontiguousarray(bnio),
        "p_lsrow": tobf(lsrow),
    }


def make_in_maps(inputs):
    x = np.ascontiguousarray(np.asarray(inputs["x"], dtype=np.float32))
    base = _pack_inputs(inputs)
    in_maps = []
    for c in range(N_CORES):
        m = dict(base)
        m["x"] = x[c * BPC:(c + 1) * BPC]
        in_maps.append(m)
    return in_maps


_NC_CACHE = None


def kernel(**inputs):
    global _NC_CACHE
    from concourse.bass_utils import run_bass_kernel_spmd

    if _NC_CACHE is None:
        _NC_CACHE = build_nc()
    nc = _NC_CACHE

    in_maps = make_in_maps(inputs)
    res = run_bass_kernel_spmd(nc, in_maps, core_ids=list(range(N_CORES)))
    out = np.concatenate([res.results[c]["out"] for c in range(N_CORES)], axis=0)
    return out.astype(np.float32)


# revision 49
# speedup vs baseline: 1.1261x; 1.1261x over previous
"""Trainium2 Bass kernel for nn_MultiHeadSelfAttentionBlock.

Data-parallel over batch (B=32 -> 4 per core on 8 cores). Single-core
pipeline, bf16 matmul operands (fp32 PSUM accumulation) throughout:

  - All weight preprocessing happens on host (numpy): BN folded to
    per-channel scale/shift, q/out projections transposed into lhsT
    layout, k/v 1x1 projections merged with the depthwise-conv taps and
    k/v BN into 45 per-(chunk,tap) [128c, 64kd_k||64kd_v] bf16 blocks
    (k-side prescaled by 1/sqrt(64)), BN-shift constants reduced to a
    [128,1] vector, layer-scale replicated to a [8,1024] row table.
  - Per item: x loaded once (kept fp32 for the residual); BN applied on
    GPSIMD writing bf16 into a zero-padded [c, 34x34] buffer.  q proj
    reads 128-pixel slices of the padded buffer as the stationary
    operand; conv taps read strided 16x16/stride-2 windows as the
    moving operand -- no im2col staging, and k+v share each matmul.
  - Logits computed transposed [p, l] per head (the torch .view
    head-split bug resolves to l = 16*c + 2*t + par, kd = s_lo); the
    softmax denominator comes free as row 64 of the o-matmul via a ones
    column appended to V^T.  exp on Scalar (bf16 out), denominator rows
    copied to SBUF on Scalar, one reciprocal per item on DVE with
    layer-scale folded in, broadcast across partitions via a DRAM
    bounce per head-pair, normalize on GPSIMD.
  - Output projection accumulates in PSUM; epilogue is a single DVE add
    of the fp32 residual.  Emission is software-pipelined: item b-1's
    output projection is emitted between taps(b) and attention(b) so
    the PE never idles on the denominator DRAM round-trip.
"""

from contextlib import ExitStack

import os

import numpy as np

import concourse.bacc as bacc
import concourse.bass as bass
import concourse.tile as tile
from concourse import mybir
from concourse.masks import make_identity
from concourse.dve_ops import RECIPROCAL_APPROX_FAST, RECIP_APPROX_FAST_CONSTS

F32 = mybir.dt.float32
BF16 = mybir.dt.bfloat16
F8 = mybir.dt.float8e4
DR = mybir.MatmulPerfMode.DoubleRow
ALU = mybir.AluOpType
ACTF = mybir.ActivationFunctionType
LS_EXP = 23                  # o_norm carries ls * 2^23 to stay in fp8 range
OW_EXP = 8                   # out_w scaled by 2^8 to avoid fp8 denormals

B, C, H, W = 32, 640, 32, 32
NH, KD, VD = 8, 64, 64
S = H * W            # 1024
P = 256              # key/value positions (16x16)
EPS = 1e-3
N_CORES = 8
BPC = B // N_CORES   # 4 batch items per core
NCH = C // 128       # 5 channel chunks
PW = 34              # padded image width
PSZ = PW * PW        # 1156


def _fap(base, free_off, dims):
    """AP with base's partition dim and explicit free dims [[step, count],...]."""
    return bass.AP(tensor=base.tensor, offset=base.offset + free_off,
                   ap=[base.ap[0]] + dims)


def build_nc():
    nc = bacc.Bacc(None, target_bir_lowering=False, debug=False)

    x4 = nc.dram_tensor("x", [BPC, C, H, W], F32, kind="ExternalInput")
    qwT_d = nc.dram_tensor("p_qwT", [128, NCH * 512], BF16, kind="ExternalInput")
    wtap_d = nc.dram_tensor("p_wtap", [128, NCH * 9 * 128], BF16,
                            kind="ExternalInput")
    owT_d = nc.dram_tensor("p_owT", [128, 4 * C], F8, kind="ExternalInput")
    kvc_d = nc.dram_tensor("p_kvconst", [128, 1], F32, kind="ExternalInput")
    bnio_d = nc.dram_tensor("p_bnio", [128, 2 * NCH], F32, kind="ExternalInput")
    lsr_d = nc.dram_tensor("p_lsrow", [128, 2 * S], BF16, kind="ExternalInput")
    out4 = nc.dram_tensor("out", [BPC, C, H, W], F32, kind="ExternalOutput")
    KSTAGE = int(os.environ.get("KSTAGE", "99"))

    with tile.TileContext(nc) as tc, ExitStack() as ctx:
        wp = ctx.enter_context(tc.tile_pool(name="wp", bufs=1))
        # single PSUM pool, tags sized to exactly 8 banks:
        #   mm 2x[128,512]f32 + kvf 1x[128,256]f32 + lg 2x[128,512]f32
        #   + op 3x[65,512]f32
        pp = ctx.enter_context(tc.tile_pool(name="pp", bufs=1, space="PSUM"))
        xin = ctx.enter_context(tc.tile_pool(name="xin", bufs=2 * NCH))
        xnfp = ctx.enter_context(tc.tile_pool(name="xnfp", bufs=NCH))
        qbp = ctx.enter_context(tc.tile_pool(name="qbp", bufs=2))
        ep = ctx.enter_context(tc.tile_pool(name="ep", bufs=4))
        kvp = ctx.enter_context(tc.tile_pool(name="kvp", bufs=2))
        orp = ctx.enter_context(tc.tile_pool(name="orp", bufs=8))
        onp = ctx.enter_context(tc.tile_pool(name="onp", bufs=4))
        rbcp = ctx.enter_context(tc.tile_pool(name="rbcp", bufs=4))
        dap = ctx.enter_context(tc.tile_pool(name="dap", bufs=2))
        osb = ctx.enter_context(tc.tile_pool(name="osb", bufs=2))
        drp = ctx.enter_context(tc.tile_pool(name="drp", bufs=2, space="DRAM"))

        # ---------------- setup ----------------
        identf = wp.tile([64, 64], F32, tag="identf", name="identf")
        make_identity(nc, identf[:])
        ones1 = wp.tile([128, 1], BF16, tag="ones1", name="ones1")
        nc.gpsimd.memset(ones1[:], 1.0)

        qwT = wp.tile([128, NCH * 512], BF16, tag="qwT", name="qwT")
        nc.sync.dma_start(out=qwT[:], in_=qwT_d[:, :])
        wtap = wp.tile([128, NCH * 9 * 128], BF16, tag="wtap", name="wtap")
        nc.sync.dma_start(out=wtap[:], in_=wtap_d[:, :])
        owT = wp.tile([128, 4 * C], F8, tag="owT", name="owT")
        nc.sync.dma_start(out=owT[:], in_=owT_d[:, :])
        kvc = wp.tile([128, 1], F32, tag="kvc", name="kvc")
        nc.sync.dma_start(out=kvc[:], in_=kvc_d[:, :])
        bnio = wp.tile([128, 2 * NCH], F32, tag="bnio", name="bnio")
        nc.sync.dma_start(out=bnio[:], in_=bnio_d[:, :])
        lsrow = wp.tile([128, 2 * S], BF16, tag="lsrow", name="lsrow")
        nc.sync.dma_start(out=lsrow[:], in_=lsr_d[:, :])

        # denominator staging: head n lives at partition 32*(n%4), column
        # block S*(n//4) (engines only address start partitions 0/32/64/96).
        dall_t = [dap.tile([128, 2 * S], F32, tag="dall", name="dall")
                  for _ in range(2)]
        for i in range(2):
            nc.gpsimd.memset(dall_t[i][:], 1.0)

        # zero-padded xn buffers: 2 item-slots x NCH chunks; borders are
        # zeroed once here and only the 32x32 interior is rewritten per item.
        xnpad = [[wp.tile([128, PSZ], BF16, tag=f"xnp{i}_{ch}",
                          name=f"xnp{i}_{ch}")
                  for ch in range(NCH)] for i in range(2)]
        for i in range(2):
            for ch in range(NCH):
                nc.gpsimd.memset(xnpad[i][ch][:], 0.0)

        def xn_interior(t):
            """interior write AP: [128, 32, 32] at offset (1,1) of 34x34."""
            return _fap(t, PW + 1, [[PW, H], [1, W]])

        def xn_tap(t, dy, dx):
            """moving conv-tap window: stride-2 16x16 -> [128c, 256p]."""
            return _fap(t, PW * dy + dx, [[2 * PW, 16], [2, 16]])

        prev = None  # (b, o_norm tiles, x tiles)

        def emit_outproj(bp, onorm_p, xt_p):
            for ch in range(NCH):
                ot = osb.tile([128, S], F32, tag="outsb", name="outsb")
                for sh in range(2):
                    po = pp.tile([128, 512], F32, tag="mm", bufs=2, name="po")
                    for nv in range(4):
                        nc.tensor.matmul(
                            po[:], owT[:, C * nv + 128 * ch:C * nv + 128 * (ch + 1)],
                            onorm_p[nv // 2][:, S * (nv % 2) + 512 * sh:
                                             S * (nv % 2) + 512 * (sh + 1)],
                            start=(nv == 0), stop=(nv == 3))
                    sl = slice(512 * sh, 512 * (sh + 1))
                    nc.vector.scalar_tensor_tensor(
                        out=ot[:, sl], in0=po[:],
                        scalar=float(2.0 ** -(LS_EXP + OW_EXP)),
                        in1=xt_p[ch][:, sl], op0=ALU.mult, op1=ALU.add)
                nc.sync.dma_start(
                    out=out4[bp, 128 * ch:128 * (ch + 1), :, :].rearrange(
                        "c h w -> c (h w)"),
                    in_=ot[:])

        # ================= per batch item =================
        for b in range(BPC):
            slot = b % 2
            # ---- load x (kept for residual), BN -> flat + padded bf16 ----
            xts, xnfs = [], []
            for ch in range(NCH):
                xt = xin.tile([128, S], F32, tag="xin", name="xin")
                nc.sync.dma_start(
                    out=xt[:],
                    in_=x4[b, 128 * ch:128 * (ch + 1), :, :].rearrange(
                        "c h w -> c (h w)"))
                xts.append(xt)
            for ch in range(NCH):
                xnf = xnfp.tile([128, S], BF16, tag="xnf", name="xnf")
                nc.gpsimd.tensor_scalar(
                    out=xnf[:], in0=xts[ch][:],
                    scalar1=bnio[:, ch:ch + 1], scalar2=bnio[:, NCH + ch:NCH + ch + 1],
                    op0=ALU.mult, op1=ALU.add)
                nc.scalar.activation(xn_interior(xnpad[slot][ch]), xnf[:],
                                     ACTF.Copy)
                xnfs.append(xnf)

            # ---- q projection -> qbuf [s%128, 512*t + c] (t-major) ----
            qbuf = qbp.tile([128, 8 * 512], BF16, tag="qbuf", name="qbuf")
            for t in range(8):
                qp = pp.tile([128, 512], F32, tag="mm", bufs=2, name="qp")
                for ch in range(NCH):
                    nc.tensor.matmul(qp[:], xnfs[ch][:, 128 * t:128 * (t + 1)],
                                     qwT[:, 512 * ch:512 * (ch + 1)],
                                     start=(ch == 0), stop=(ch == NCH - 1))
                nc.vector.tensor_copy(qbuf[:, 512 * t:512 * (t + 1)], qp[:])

            if KSTAGE == 1:
                nc.sync.dma_start(
                    out=out4[b, 0:128, :, :].rearrange("c h w -> c (h w)"),
                    in_=qbuf[:, 0:1024].bitcast(F32))
                continue

            # ---- merged k|v conv taps -> kvf PSUM [64kf || 64vf, 256] ----
            kvf = pp.tile([128, 256], F32, tag="mm", bufs=2, name="kvf")
            for ch in range(NCH):
                for t in range(9):
                    nc.tensor.matmul(
                        kvf[:],
                        wtap[:, 128 * (9 * ch + t):128 * (9 * ch + t + 1)],
                        xn_tap(xnpad[slot][ch], t // 3, t % 3),
                        start=(ch == 0 and t == 0),
                        stop=(ch == NCH - 1 and t == 8))
            kfdup = kvp.tile([128, 256], BF16, tag="f_k", name="f_k")
            nc.vector.tensor_scalar_add(kfdup[0:64, :], kvf[0:64, :],
                                        kvc[0:64, :])
            nc.vector.tensor_scalar_add(kfdup[64:128, :], kvf[0:64, :],
                                        kvc[0:64, :])
            vf = kvp.tile([64, 256], F32, tag="f_v", name="f_v")
            nc.vector.tensor_scalar_add(vf[:], kvf[64:128, :], kvc[64:128, :])

            # V'^T with ones column, fp8; p-tiles at 16-aligned stride 80
            # (DoubleRow requires the pair-dim step % 16 == 0)
            vT8 = kvp.tile([128, 2 * 80], F8, tag="vT8", name="vT8")
            for pt in range(2):
                tp = pp.tile([128, 512], F32, tag="mm", bufs=2, name="tp")
                nc.tensor.transpose(tp[:128, 0:64],
                                    vf[:, 128 * pt:128 * (pt + 1)],
                                    identf[0:64, 0:64])
                nc.scalar.activation(vT8[:, 80 * pt:80 * pt + 64],
                                     tp[:128, 0:64], ACTF.Copy)
                nc.vector.tensor_copy(vT8[:, 80 * pt + 64:80 * pt + 65],
                                      ones1[:])

            if KSTAGE == 2:
                nc.sync.dma_start(
                    out=out4[b, 0:128, 0:4, :].rearrange("c h w -> c (h w)"),
                    in_=kfdup[:, :].bitcast(F32))
                continue

            # ---- previous item's output projection (pipelined) ----
            if prev is not None:
                emit_outproj(*prev)
                prev = None

            # ---- attention ----
            dall = dall_t[slot]
            o_resh = [orp.tile([128, S], BF16, tag="oresh", name="oresh")
                      for _ in range(4)]
            rec = dap.tile([128, 2 * S], F32, tag="rec", bufs=1, name="rec")
            dall2 = dap.tile([128, 2 * S], BF16, tag="dall2", name="dall2")
            dscr = drp.tile([NH, S], BF16, tag="dscr", name="dscr")
            o_norm = []
            rbcs = []

            def denom_halfbatch(blk):
                # reciprocal + layer scale for heads 4*blk..4*blk+3, then
                # bounce to DRAM and normalize the two finished c2 blocks
                cs = slice(S * blk, S * (blk + 1))
                nc.vector._custom_dve(
                    RECIPROCAL_APPROX_FAST, out=rec[:, cs], in0=dall[:, cs],
                    s0=RECIP_APPROX_FAST_CONSTS["s0"],
                    s1=RECIP_APPROX_FAST_CONSTS["s1"],
                    imm2=RECIP_APPROX_FAST_CONSTS["imm2"])
                nc.vector.tensor_tensor(out=dall2[:, cs], in0=rec[:, cs],
                                        in1=lsrow[:, cs], op=ALU.mult)
                for m in range(4):
                    nc.sync.dma_start(
                        out=dscr[4 * blk + m:4 * blk + m + 1, :],
                        in_=dall2[32 * m:32 * m + 1, cs])
                # one fp8 pair-tile per half: cols [0:S] = c2 even, [S:2S] odd
                on = onp.tile([128, 2 * S], F8, tag="onorm", name="onorm")
                for c2 in (2 * blk, 2 * blk + 1):
                    rbc = rbcp.tile([128, S], BF16, tag="rbc", name="rbc")
                    rbcs.append(rbc)
                    nc.sync.dma_start(
                        out=rbc[:],
                        in_=bass.AP(tensor=dscr.tensor,
                                    offset=dscr.offset + S * 2 * c2,
                                    ap=[[S, 2], [0, 64], [1, S]]))
                    nc.vector.tensor_tensor(
                        out=on[:, S * (c2 % 2):S * (c2 % 2 + 1)],
                        in0=o_resh[c2][:], in1=rbc[:], op=ALU.mult)
                o_norm.append(on)

            for n in range(NH):
                E8 = ep.tile([128, 2 * S], F8, tag="E", name="E")
                for pt in range(2):
                    for par in range(2):
                        lg = pp.tile([128, 512], F32, tag="lg", bufs=2,
                                     name="lg")
                        # head n's q: cols {512*t + 64*n + c'} of qbuf
                        rhs = _fap(qbuf[64 * par:64 * (par + 1)], 64 * n,
                                   [[512, 8], [1, 64]])
                        nc.tensor.matmul(
                            lg[:],
                            kfdup[64 * par:64 * (par + 1),
                                  128 * pt:128 * (pt + 1)],
                            rhs, start=True, stop=True)
                        nc.scalar.activation(
                            E8[:, S * pt + 512 * par:S * pt + 512 * (par + 1)],
                            lg[:], ACTF.Exp)
                # o via DoubleRow over the two p-tiles: op_t cols =
                # 512*par + j', j' = 64*t + c' (l = 16*c' + 2*t + par)
                op_t = pp.tile([65, 2 * 512], F32, tag="op", bufs=2, name="op")
                for par in range(2):
                    lhsT = bass.AP(tensor=vT8.tensor, offset=vT8.offset,
                                   ap=[vT8.ap[0], [80, 2], [1, 65]])
                    nc.tensor.matmul(
                        op_t[:, 512 * par:512 * (par + 1)], lhsT,
                        _fap(E8[:], 512 * par, [[S, 2], [1, 512]]),
                        perf_mode=DR, start=True, stop=True)
                # un-permute to s-order on the write: src col 512*par + j
                # -> dst col 2*j + par  (s = l = 2*j + par)
                nc.scalar.activation(
                    _fap(dall[32 * (n % 4):32 * (n % 4) + 1], S * (n // 4),
                         [[1, 2], [2, 512]]),
                    op_t[64:65, :], ACTF.Copy)
                nc.vector.tensor_copy(
                    _fap(o_resh[n // 2][64 * (n % 2):64 * (n % 2) + 64], 0,
                         [[1, 2], [2, 512]]),
                    op_t[0:64, :])
                if n % 4 == 3:
                    denom_halfbatch(n // 4)

            if KSTAGE == 4:
                nc.sync.dma_start(
                    out=out4[b, 0:128, 0:16, :].rearrange("c h w -> c (h w)"),
                    in_=dall[:, 0:512])
                nc.sync.dma_start(
                    out=out4[b, 0:128, 16:32, :].rearrange("c h w -> c (h w)"),
                    in_=dall[:, 1024:1536])
                nc.sync.dma_start(
                    out=out4[b, 128:256, 0:16, :].rearrange("c h w -> c (h w)"),
                    in_=rec[:, 0:512])
                nc.sync.dma_start(
                    out=out4[b, 128:256, 16:32, :].rearrange("c h w -> c (h w)"),
                    in_=rec[:, 1024:1536])
                nc.sync.dma_start(
                    out=out4[b, 256:384, 0:16, :].rearrange("c h w -> c (h w)"),
                    in_=o_resh[1][:, :].bitcast(F32))
                nc.sync.dma_start(
                    out=out4[b, 384:512, 0:32, :].rearrange("c h w -> c (h w)"),
                    in_=dall2[:, :].bitcast(F32))
                nc.sync.dma_start(
                    out=out4[b, 512:640, 0:16, :].rearrange("c h w -> c (h w)"),
                    in_=rbcs[1][:, :].bitcast(F32))
                continue
            if KSTAGE == 3:
                for j in range(2):
                    nc.sync.dma_start(
                        out=out4[b, 128 * j:128 * (j + 1), 0:16, :].rearrange(
                            "c h w -> c (h w)"),
                        in_=o_norm[j][:, :].bitcast(F32))
                continue
            prev = (b, o_norm, xts)

        if prev is not None:
            emit_outproj(*prev)

    nc.finalize()
    return nc


def _pack_inputs(inputs):
    """Host-side weight folding: everything that doesn't depend on x."""
    import ml_dtypes

    f32 = lambda n: np.asarray(inputs[n], dtype=np.float32)
    bnf = {}
    for p in ("in", "k", "v"):
        sc = f32(f"{p}_bn_gamma") / np.sqrt(f32(f"{p}_bn_var") + EPS)
        sh = f32(f"{p}_bn_beta") - f32(f"{p}_bn_mean") * sc
        bnf[p] = (sc, sh)

    def tobf(a):
        return np.ascontiguousarray(a.astype(ml_dtypes.bfloat16))

    def tof8(a):
        return np.ascontiguousarray(a.astype(mybir.dt.np(mybir.dt.float8e4)))

    q_w = f32("q_w")                     # [512, 640]
    qwT = np.zeros((128, NCH * 512), np.float32)
    for ch in range(NCH):
        qwT[:, 512 * ch:512 * (ch + 1)] = q_w[:, 128 * ch:128 * (ch + 1)].T

    sck, shk = bnf["k"]
    scv, shv = bnf["v"]
    kw_s = f32("k_w") * sck[None, :] * 0.125      # [64, 640]
    vw_s = f32("v_w") * scv[None, :]
    kdw = f32("k_dw_w").reshape(C, 9)
    vdw = f32("v_dw_w").reshape(C, 9)
    wtap = np.zeros((128, NCH * 9 * 128), np.float32)
    for ch in range(NCH):
        cs = slice(128 * ch, 128 * (ch + 1))
        for t in range(9):
            blk = wtap[:, 128 * (9 * ch + t):128 * (9 * ch + t + 1)]
            blk[:, 0:64] = kw_s[:, cs].T * kdw[cs, t][:, None]
            blk[:, 64:128] = vw_s[:, cs].T * vdw[cs, t][:, None]

    kvconst = np.zeros((128, 1), np.float32)
    kvconst[0:64, 0] = (kw_s @ shk)
    kvconst[64:128, 0] = (vw_s @ shv)

    out_w = f32("out_w") * float(2.0 ** 8)   # fp8-friendly scale
    owT = np.zeros((128, 4 * C), np.float32)
    for nv in range(4):
        owT[:, C * nv:C * (nv + 1)] = out_w[:, 128 * nv:128 * (nv + 1)].T

    sci, shi = bnf["in"]
    bnio = np.zeros((128, 2 * NCH), np.float32)
    for ch in range(NCH):
        bnio[:, ch] = sci[128 * ch:128 * (ch + 1)]
        bnio[:, NCH + ch] = shi[128 * ch:128 * (ch + 1)]

    ls = f32("ls_gamma") * float(2.0 ** 23)  # fp8-friendly scale for o_norm
    lsrow = np.zeros((128, 2 * S), np.float32)
    for n in range(NH):
        lsrow[32 * (n % 4), S * (n // 4):S * (n // 4 + 1)] = np.tile(ls, H)

    return {
        "p_qwT": tobf(qwT),
        "p_wtap": tobf(wtap),
        "p_owT": tof8(owT),
        "p_kvconst": np.ascontiguousarray(kvconst),
        "p_bnio": np.ascontiguousarray(bnio),
        "p_lsrow": tobf(lsrow),
    }


def make_in_maps(inputs):
    x = np.ascontiguousarray(np.asarray(inputs["x"], dtype=np.float32))
    base = _pack_inputs(inputs)
    in_maps = []
    for c in range(N_CORES):
        m = dict(base)
        m["x"] = x[c * BPC:(c + 1) * BPC]
        in_maps.append(m)
    return in_maps


_NC_CACHE = None


def kernel(**inputs):
    global _NC_CACHE
    from concourse.bass_utils import run_bass_kernel_spmd

    if _NC_CACHE is None:
        _NC_CACHE = build_nc()
    nc = _NC_CACHE

    in_maps = make_in_maps(inputs)
    res = run_bass_kernel_spmd(nc, in_maps, core_ids=list(range(N_CORES)))
    out = np.concatenate([res.results[c]["out"] for c in range(N_CORES)], axis=0)
    return out.astype(np.float32)
